# revision 6
# baseline (speedup 1.0000x reference)
"""Differential attention kernel for Trainium2 (8 NeuronCores, batch-parallel).

Reference computation (per batch b):
    Q = X @ W_q + b_q ; K = X @ W_k + b_k ; V = X @ W_v + b_v
    A_i = (Q_i @ K_i^T) / sqrt(D)          (i = 1, 2 halves of the 2D dim)
    P   = softmax(A_1) - lam * softmax(A_2)
    out = P @ V
Shapes: B=8, S=2048, E=1024, D=512.  One batch per NeuronCore.

v3 strategy (per core):
  - Host pre-transposes X -> X^T [E, S] and converts X^T / W_q / W_k / W_v
    to bf16 (outputs were already stored bf16, so matmul-input rounding
    adds ~nothing; halves DMA and speeds the PE vs fp32r).
  - Phase A (single pass over X^T): Q^T, K^T (+biases via ACT) and V (DVE
    copy) all RESIDENT in SBUF as bf16 - no DRAM spill round-trip.
  - Phase B: per 512-query block, branch-sequential: scores^T =
    K^T-slices.T @ Q^T-block (contraction dim on partitions) in bf16, exp
    via ACT into 2-bank PSUM pairs.  V is stored ones-AUGMENTED as
    [KT, 2, 257] (two 256-wide halves each with a trailing 1.0 column), so
    the PV matmul itself produces the softmax row-sums in PSUM column 256
    with queries on partitions: no separate rowsum matmuls, no 32x32
    transposes.  DVE reciprocal on that column -> per-partition scale;
    branch 0 evicts via ACT (scale), branch 1 fuses (out -= lam/r2 * PV)
    into one DVE scalar_tensor_tensor per half (lam pre-negated on host).
    Two full ea-tile sets (one per branch) so branch 1's exp evictions
    don't serialize behind branch 0's PV reads.
  - softmax max-subtraction is skipped (scores are O(1); exp is safe).
  - V bias folded on host: P rows sum to (1-lam), so out += (1-lam)*b_v.
  - dyn_rep=True builds a timing variant whose repeat count is read at
    runtime from a [1,1] int32 input, so one compile serves all R.
"""

import math
from contextlib import ExitStack

import numpy as np

import concourse.bass as bass
import concourse.tile as tile
from concourse import bacc, mybir
from concourse import bass_utils

F32 = mybir.dt.float32
I32 = mybir.dt.int32
BF16 = mybir.dt.bfloat16
AFT = mybir.ActivationFunctionType
ALU = mybir.AluOpType

P = 128
B, S, E, D = 8, 2048, 1024, 512
EC = E // P          # 8 e-chunks
DT = (2 * D) // P    # 8 d-tiles over the 2D projection dim
KT = S // P          # 16 k-tiles
SBLK = 4             # phase-A s-blocks of 512
SB = S // SBLK       # 512
QB = 4               # phase-B q-blocks of 512
QBS = S // QB        # 512
QS = QBS // P        # 4 q-subtiles per q-block
DH = D // 2          # 256: V half-width (augmented to 257 with ones col)
SCALE = 1.0 / math.sqrt(D)

_NC_CACHE = {}


def _build_nc(repeat=1, loop_scope="all", ablate=(), dyn_rep=False):
    # ablate: subset of {"evict", "dma"} - timing experiments only
    nc = bacc.Bacc("TRN2", target_bir_lowering=False, debug=False)

    xt_d = nc.dram_tensor("xt", [EC, P, S], BF16, kind="ExternalInput").ap()
    wq_d = nc.dram_tensor("wq", [EC, P, 2 * D], BF16, kind="ExternalInput").ap()
    wk_d = nc.dram_tensor("wk", [EC, P, 2 * D], BF16, kind="ExternalInput").ap()
    wv_d = nc.dram_tensor("wv", [EC, P, D], BF16, kind="ExternalInput").ap()
    bq_d = nc.dram_tensor("bq", [P, DT], F32, kind="ExternalInput").ap()
    bk_d = nc.dram_tensor("bk", [P, DT], F32, kind="ExternalInput").ap()
    lam_d = nc.dram_tensor("lam128", [P, 1], F32, kind="ExternalInput").ap()
    if dyn_rep:
        rep_d = nc.dram_tensor("rep", [1, 1], I32, kind="ExternalInput").ap()
    out_d = nc.dram_tensor("out", [KT, P, D], F32, kind="ExternalOutput").ap()

    with tile.TileContext(nc) as tc, ExitStack() as ctx:
        rep_ctx = ExitStack()
        const = ctx.enter_context(tc.tile_pool(name="const", bufs=1))
        bq_sb = const.tile([P, DT], F32)
        nc.sync.dma_start(bq_sb, bq_d)
        bk_sb = const.tile([P, DT], F32)
        nc.sync.dma_start(bk_sb, bk_d)
        lam_sb = const.tile([P, 1], F32)   # holds -lam
        nc.sync.dma_start(lam_sb, lam_d)
        if dyn_rep:
            rep_sb = const.tile([1, 1], I32)
            nc.sync.dma_start(rep_sb, rep_d)
            rep_val = nc.values_load(
                rep_sb[0:1, 0:1], min_val=1, max_val=1 << 20,
                skip_runtime_bounds_check=True)

        persist = ctx.enter_context(tc.tile_pool(name="persist", bufs=1))
        kt_sb = persist.tile([P, DT, S], BF16, tag="kt")
        v_sb = persist.tile([P, KT, 2, DH + 1], BF16, tag="v")
        qt_sb = persist.tile([P, DT, S], BF16, tag="qt")
        # ones-augment columns, written once (phase A only writes [:, :, :, 0:DH])
        nc.vector.memset(v_sb[:, :, :, DH:DH + 1], 1.0)

        if dyn_rep and loop_scope in ("all", "A"):
            rep_ctx.enter_context(tc.For_i(0, rep_val, 1))
        elif not dyn_rep and repeat > 1 and loop_scope in ("all", "A"):
            rep_ctx.enter_context(tc.For_i(0, repeat, 1))

        # ------------- Phase A: projections (single pass over X^T) -------------
        with tc.tile_pool(name="wp", bufs=1) as wp, \
             tc.tile_pool(name="xtp", bufs=2) as xtp, \
             tc.tile_pool(name="psA", bufs=4, space="PSUM") as psA:
            wq_sb = wp.tile([P, EC, 2 * D], BF16)
            wk_sb = wp.tile([P, EC, 2 * D], BF16)
            wv_sb = wp.tile([P, EC, D], BF16)
            # interleave critical-path loads: wq_dt0, xt(sblk0), wv, wq rest, wk
            nc.sync.dma_start(
                wq_sb[:, :, 0:P],
                wq_d[:, :, 0:P].rearrange("c p d -> p c d"))
            xt_first = xtp.tile([P, EC, SB], BF16, tag="xt", name="xt_first")
            for ec in range(EC):
                eng = nc.sync if ec % 2 == 0 else nc.scalar
                eng.dma_start(xt_first[:, ec, :], xt_d[ec, :, 0:SB])
            for ec in range(EC):
                nc.scalar.dma_start(wv_sb[:, ec, :], wv_d[ec])
            for dt in range(1, DT):
                eng = nc.sync if dt % 2 == 0 else nc.scalar
                eng.dma_start(
                    wq_sb[:, :, P * dt:P * (dt + 1)],
                    wq_d[:, :, P * dt:P * (dt + 1)].rearrange("c p d -> p c d"))
            for dt in range(DT):
                eng = nc.sync if dt % 2 == 0 else nc.scalar
                eng.dma_start(
                    wk_sb[:, :, P * dt:P * (dt + 1)],
                    wk_d[:, :, P * dt:P * (dt + 1)].rearrange("c p d -> p c d"))

            for sblk in range(SBLK):
                if sblk == 0:
                    xt_t = xt_first
                else:
                    xt_t = xtp.tile([P, EC, SB], BF16, tag="xt")
                    nld = 8 if "dma" in ablate else SB
                    for ec in range(EC):
                        eng = nc.sync if ec % 2 == 0 else nc.scalar
                        eng.dma_start(
                            xt_t[:, ec, 0:nld],
                            xt_d[ec, :, SB * sblk:SB * sblk + nld])
                # Q^T resident (bf16, +bias via ACT)
                for dt in range(DT):
                    ps = psA.tile([P, SB], F32, tag="ps")
                    for ec in range(EC):
                        nc.tensor.matmul(
                            ps, wq_sb[:, ec, P * dt:P * (dt + 1)], xt_t[:, ec, :],
                            start=(ec == 0), stop=(ec == EC - 1))
                    ev = 8 if "evict" in ablate else SB
                    nc.scalar.activation(
                        qt_sb[:, dt, SB * sblk:SB * sblk + ev], ps[:, 0:ev],
                        AFT.Identity, bias=bq_sb[:, dt:dt + 1], scale=1.0)
                # V resident (bf16 via DVE, ones-augmented, bias folded on host)
                for kt4 in range(SB // P):
                    kti = (SB // P) * sblk + kt4
                    ps = psA.tile([P, D], F32, tag="ps")
                    for ec in range(EC):
                        nc.tensor.matmul(
                            ps, xt_t[:, ec, P * kt4:P * (kt4 + 1)], wv_sb[:, ec, :],
                            start=(ec == 0), stop=(ec == EC - 1))
                    ev = 8 if "evict" in ablate else DH
                    for h in range(2):
                        nc.vector.tensor_copy(v_sb[:, kti, h, 0:ev],
                                              ps[:, DH * h:DH * h + ev])
                # K^T resident (bf16, +bias via ACT)
                for dt in range(DT):
                    ps = psA.tile([P, SB], F32, tag="ps")
                    for ec in range(EC):
                        nc.tensor.matmul(
                            ps, wk_sb[:, ec, P * dt:P * (dt + 1)], xt_t[:, ec, :],
                            start=(ec == 0), stop=(ec == EC - 1))
                    ev = 8 if "evict" in ablate else SB
                    nc.scalar.activation(
                        kt_sb[:, dt, SB * sblk:SB * sblk + ev], ps[:, 0:ev],
                        AFT.Identity, bias=bk_sb[:, dt:dt + 1], scale=1.0)

        if loop_scope == "A" and (dyn_rep or repeat > 1):
            rep_ctx.close()
        if loop_scope == "B":
            if dyn_rep:
                rep_ctx.enter_context(tc.For_i(0, rep_val, 1))
            elif repeat > 1:
                rep_ctx.enter_context(tc.For_i(0, repeat, 1))

        # ---------------- Phase B: attention ----------------
        # Branch-sequential per q-block; expA tiles hold k-tile PAIRS
        # ([128, 1024], two PSUM banks) so one ACT eviction serves 8 matmuls.
        # Two ea tile sets (one per branch) avoid br0->br1 serialization.
        with tc.tile_pool(name="eap", bufs=1) as eap, \
             tc.tile_pool(name="rsp", bufs=4) as rsp, \
             tc.tile_pool(name="outp", bufs=2) as outp, \
             tc.tile_pool(name="psS", bufs=2, space="PSUM") as psS, \
             tc.tile_pool(name="psO", bufs=2, space="PSUM") as psO:
            for qb in range(QB):
                osb1 = {}
                for br in range(2):
                    ea = {}
                    for kp in range(KT // 2):       # k-tile pairs
                        ps = psS.tile([P, 2 * QBS], F32, tag="ps_s")
                        for half in range(2):
                            kt = 2 * kp + half
                            for dch in range(4):
                                dt = 4 * br + dch
                                nc.tensor.matmul(
                                    ps[:, QBS * half:QBS * (half + 1)],
                                    kt_sb[:, dt, P * kt:P * (kt + 1)],
                                    qt_sb[:, dt, QBS * qb:QBS * (qb + 1)],
                                    start=(dch == 0), stop=(dch == 3))
                        t = eap.tile([P, 2 * QBS], BF16, tag=f"ea{br}_{kp}",
                                     name=f"ea{br}_{kp}")
                        if "evict" in ablate:
                            nc.scalar.activation(t[:, 0:8], ps[:, 0:8], AFT.Exp,
                                                 scale=SCALE)
                        else:
                            nc.scalar.activation(t, ps, AFT.Exp, scale=SCALE)
                        ea[kp] = t
                    for qs in range(QS):
                        pso = psO.tile([P, 2, 2 * DH], F32, tag="ps_o")
                        for kt in range(KT):
                            eas = ea[kt // 2][:, QBS * (kt % 2) + P * qs:
                                              QBS * (kt % 2) + P * (qs + 1)]
                            for h in range(2):
                                nc.tensor.matmul(
                                    pso[:, h, 0:DH + 1], eas,
                                    v_sb[:, kt, h, :],
                                    start=(kt == 0), stop=(kt == KT - 1))
                        rt = rsp.tile([P, 1], F32, tag="rt")
                        nc.vector.reciprocal(rt, pso[:, 0, DH:DH + 1])
                        ev = 8 if "evict" in ablate else DH
                        if br == 0:
                            o = outp.tile([P, D], F32, tag=f"osb1_{qs}")
                            for h in range(2):
                                nc.scalar.activation(
                                    o[:, DH * h:DH * h + ev], pso[:, h, 0:ev],
                                    AFT.Copy, scale=rt)
                            osb1[qs] = o
                        else:
                            # osb1 -= lam/r2 * PV  (lam_sb holds -lam)
                            nc.vector.tensor_scalar_mul(rt, in0=rt,
                                                        scalar1=lam_sb)
                            for h in range(2):
                                sl = slice(DH * h, DH * h + ev)
                                nc.vector.scalar_tensor_tensor(
                                    osb1[qs][:, sl], pso[:, h, 0:ev], rt,
                                    osb1[qs][:, sl], op0=ALU.mult, op1=ALU.add)
                            odma = 8 if "evict" in ablate else D
                            nc.scalar.dma_start(out_d[QS * qb + qs, :, 0:odma],
                                                osb1[qs][:, 0:odma])

        rep_ctx.close()

    nc.compile()
    return nc


def _get_nc():
    if "nc" not in _NC_CACHE:
        _NC_CACHE["nc"] = _build_nc()
    return _NC_CACHE["nc"]


def _marshal(X, lam_f, W_q, b_q, W_k, b_k, W_v):
    import ml_dtypes
    BF = ml_dtypes.bfloat16
    wq_r = np.ascontiguousarray(W_q.reshape(EC, P, 2 * D)).astype(BF)
    wk_r = np.ascontiguousarray(W_k.reshape(EC, P, 2 * D)).astype(BF)
    wv_r = np.ascontiguousarray(W_v.reshape(EC, P, D)).astype(BF)
    bq_r = np.ascontiguousarray(b_q.reshape(DT, P).T)
    bk_r = np.ascontiguousarray(b_k.reshape(DT, P).T)
    lam128 = np.full((P, 1), -lam_f, np.float32)   # pre-negated
    in_maps = []
    for i in range(B):
        xt_i = np.ascontiguousarray(X[i].T).astype(BF).reshape(EC, P, S)
        in_maps.append({
            "xt": xt_i, "wq": wq_r, "wk": wk_r, "wv": wv_r,
            "bq": bq_r, "bk": bk_r, "lam128": lam128,
        })
    return in_maps


def kernel(X, lam, W_q, b_q, W_k, b_k, W_v, b_v):
    X = np.asarray(X, dtype=np.float32)
    lam_f = float(np.asarray(lam))
    W_q = np.asarray(W_q, dtype=np.float32)
    b_q = np.asarray(b_q, dtype=np.float32)
    W_k = np.asarray(W_k, dtype=np.float32)
    b_k = np.asarray(b_k, dtype=np.float32)
    W_v = np.asarray(W_v, dtype=np.float32)
    b_v = np.asarray(b_v, dtype=np.float32)

    in_maps = _marshal(X, lam_f, W_q, b_q, W_k, b_k, W_v)
    nc = _get_nc()
    res = bass_utils.run_bass_kernel_spmd(nc, in_maps, core_ids=list(range(B)))

    vbias = (1.0 - lam_f) * b_v  # P rows sum to (1-lam): fold V bias here
    out = np.empty((B, S, D), np.float32)
    for i in range(B):
        out[i] = res.results[i]["out"].reshape(S, D) + vbias
    return out
